# revision 1
# baseline (speedup 1.0000x reference)
"""Trainium2 Bass kernel for nn_DVAT_5403068858731 (retrieval_knn).

Algorithm (mathematically identical to the reference, validated offline):
  The reference builds an "allowed" mask from the top-8 of pred_lm and takes
  argmax over dir_dot_grad restricted to allowed & non-special & != src_token.
  Since only <=8 vocab positions per (b,s) are allowed, we never materialize
  the [B,S,V] einsums: we find the top-8 of pred_lm per row, gather the 8
  embedding rows, and compute the 8 dir_dot_grad values directly.

  Top-8 over V=30522 (f32-exact, matching jax tie semantics):
    1. pool_max over 120 segments of 256 (single streaming DVE pass)
    2. max8 + max_index over segment maxes -> top-8 segments
    3. sort the 8 segment ids ascending (so gathered data is in ascending
       global column order -> first-occurrence ties match jax.top_k)
    4. indirect-DMA re-gather of the 8 winning segments (f32)
    5. max8 + max_index over the gathered 2048 values -> exact top-8

Sharding: data-parallel over the 2048 (b,s) rows, 256 rows per core;
embedding_matrix replicated (only gathered rows are ever read).
"""

import numpy as np

import concourse.bass as bass
import concourse.bacc as bacc
import concourse.mybir as mybir
from concourse.bass import IndirectOffsetOnAxis
from concourse.tile import TileContext

# problem constants (hardcoded per harness contract)
B, S, V, D = 4, 512, 30522, 768
N_CORES = 8
ROWS = B * S                 # 2048
R = ROWS // N_CORES          # 256 rows per core
P = 128                      # partitions
T = R // P                   # 2 row-tiles per core
L = 128                      # segment length
G = 240                      # segments per row
VPAD = L * G                 # 30720
CW = 5120                    # streamed chunk width
NCHUNK = VPAD // CW          # 6
K = 8                        # candidates (TOPK)
DP = D + 1                   # embedding row + its squared norm
NEG = float(np.float32(-3.0e38))
NUM_SPECIAL = 999
SWAP_THRESH = float(np.float32(0.7))   # 1.0 - swap_ratio in f32

f32 = mybir.dt.float32
i32 = mybir.dt.int32
u32 = mybir.dt.uint32
Alu = mybir.AluOpType


def _pool_max(nc, out, in3):
    """Segmented max: reduce the innermost axis of a [p, g, l] AP.
    (InstPool doesn't exist on TRN2; InstTensorReduce does.)"""
    return nc.vector.reduce_max(out=out, in_=in3, axis=mybir.AxisListType.X)


def build_nc():
    nc = bacc.Bacc()
    pred = nc.dram_tensor("pred", [R, VPAD], f32, kind="ExternalInput")
    dg = nc.dram_tensor("dg", [R, D], f32, kind="ExternalInput")
    se = nc.dram_tensor("se", [R, D], f32, kind="ExternalInput")
    embp = nc.dram_tensor("embp", [V, DP], f32, kind="ExternalInput")
    tokf = nc.dram_tensor("tokf", [R, 1], f32, kind="ExternalInput")
    ruf = nc.dram_tensor("ruf", [R, 1], f32, kind="ExternalInput")
    amf = nc.dram_tensor("amf", [R, 1], f32, kind="ExternalInput")
    rbf = nc.dram_tensor("rbf", [R, 1], f32, kind="ExternalInput")
    adv = nc.dram_tensor("adv", [R, 1], f32, kind="ExternalOutput")

    pred_flat = pred[:, :].rearrange("a (g l) -> (a g) l", l=L)  # [R*G, L]

    with TileContext(nc) as tc:
        with (
            tc.tile_pool(name="pp", bufs=3) as pp,      # streamed pred chunks
            tc.tile_pool(name="gp", bufs=2) as gp,      # gathered data
            tc.tile_pool(name="mp", bufs=2) as mp,      # small working tiles
            tc.tile_pool(name="cp", bufs=1) as cp,      # constants
        ):
            # constant tables for the fused pos->(kslot,off)->col decode:
            # thresh[p, k, j] = 256*(j+1);  jconst[p, k, j] = j   (j inner)
            thresh_i = cp.tile([P, K * K], i32, tag="thresh_i")
            nc.gpsimd.iota(thresh_i[:, :], [[0, K], [L, K]], base=L,
                           channel_multiplier=0)
            thresh = cp.tile([P, K * K], f32, tag="thresh")
            nc.vector.tensor_copy(out=thresh[:, :], in_=thresh_i[:, :])
            jconst_i = cp.tile([P, K * K], i32, tag="jconst_i")
            nc.gpsimd.iota(jconst_i[:, :], [[0, K], [1, K]], base=0,
                           channel_multiplier=0)
            jconst = cp.tile([P, K * K], f32, tag="jconst")
            nc.vector.tensor_copy(out=jconst[:, :], in_=jconst_i[:, :])

            for t in range(T):
                rows = slice(t * P, (t + 1) * P)

                # ---- Phase A: stream pred, per-segment max ----
                segmax = mp.tile([P, G], f32, tag="segmax")
                for j in range(NCHUNK):
                    pt = pp.tile([P, CW], f32, tag="pred")
                    nc.sync.dma_start(
                        out=pt[:, :], in_=pred[rows, j * CW:(j + 1) * CW]
                    )
                    _pool_max(
                        nc,
                        out=segmax[:, j * (CW // L):(j + 1) * (CW // L)],
                        in3=pt[:, :].rearrange("p (g l) -> p g l", l=L),
                    )

                # ---- Phase B: rank segments, top-8 ----
                sm8 = mp.tile([P, K], f32, tag="sm8")
                sidx = mp.tile([P, K], u32, tag="sidx")
                nc.vector.max(out=sm8[:, :], in_=segmax[:, :])
                nc.vector.max_index(
                    out=sidx[:, :], in_max=sm8[:, :], in_values=segmax[:, :]
                )

                # ---- Phase C: sort seg ids ascending; flat gather rows ----
                sidxf = mp.tile([P, K], f32, tag="sidxf")
                nc.vector.tensor_copy(out=sidxf[:, :], in_=sidx[:, :])
                snegf = mp.tile([P, K], f32, tag="snegf")
                nc.vector.tensor_scalar_mul(snegf[:, :], sidxf[:, :], -1.0)
                ssortn = mp.tile([P, K], f32, tag="ssortn")
                nc.vector.max(out=ssortn[:, :], in_=snegf[:, :])
                ssortf = mp.tile([P, K], f32, tag="ssortf")   # ascending seg ids
                nc.vector.tensor_scalar_mul(ssortf[:, :], ssortn[:, :], -1.0)

                rb_t = mp.tile([P, 1], f32, tag="rb")
                nc.sync.dma_start(out=rb_t[:, :], in_=rbf[rows, :])
                flati = mp.tile([P, K], i32, tag="flati")
                nc.vector.tensor_scalar_add(flati[:, :], ssortf[:, :], rb_t[:, :1])

                # ---- Phase D: gather the 8 winning segments (f32 exact) ----
                cand = gp.tile([P, K * L], f32, tag="cand")
                for k in range(K):
                    nc.gpsimd.indirect_dma_start(
                        out=cand[:, k * L:(k + 1) * L],
                        out_offset=None,
                        in_=pred_flat,
                        in_offset=IndirectOffsetOnAxis(
                            ap=flati[:, k:k + 1], axis=0
                        ),
                    )

                # ---- Phase E: exact top-8 within gathered data ----
                v8 = mp.tile([P, K], f32, tag="v8")
                pos = mp.tile([P, K], u32, tag="pos")
                nc.vector.max(out=v8[:, :], in_=cand[:, :])
                nc.vector.max_index(out=pos[:, :], in_max=v8[:, :], in_values=cand[:, :])

                # decode pos -> (kslot, off) -> global column, fused via the
                # constant j-tables: kslot = sum_j [pos >= 256(j+1)],
                # segsel = sum_j [kslot == j] * ssortf[j], col = segsel*L + off
                posf = mp.tile([P, K], f32, tag="posf")
                nc.vector.tensor_copy(out=posf[:, :], in_=pos[:, :])
                posb = posf[:, :].rearrange(
                    "p (k o) -> p k o", o=1
                ).to_broadcast([P, K, K])
                cmpkj = mp.tile([P, K * K], f32, tag="cmpkj")
                cmp3 = cmpkj[:, :].rearrange("p (k j) -> p k j", j=K)
                nc.vector.tensor_tensor(
                    out=cmp3, in0=posb,
                    in1=thresh[:, :].rearrange("p (k j) -> p k j", j=K),
                    op=Alu.is_ge,
                )
                kslotf = mp.tile([P, K], f32, tag="kslotf")
                nc.vector.reduce_sum(
                    out=kslotf[:, :], in_=cmp3, axis=mybir.AxisListType.X
                )
                offf = mp.tile([P, K], f32, tag="offf")
                nc.vector.tensor_scalar_mul(offf[:, :], kslotf[:, :], -float(L))
                nc.vector.tensor_tensor(
                    out=offf[:, :], in0=offf[:, :], in1=posf[:, :], op=Alu.add
                )
                kslotb = kslotf[:, :].rearrange(
                    "p (k o) -> p k o", o=1
                ).to_broadcast([P, K, K])
                nc.vector.tensor_tensor(
                    out=cmp3, in0=kslotb,
                    in1=jconst[:, :].rearrange("p (k j) -> p k j", j=K),
                    op=Alu.is_equal,
                )
                nc.vector.tensor_tensor(
                    out=cmp3, in0=cmp3,
                    in1=ssortf[:, :].rearrange(
                        "p (o j) -> p o j", o=1
                    ).to_broadcast([P, K, K]),
                    op=Alu.mult,
                )
                segsel = mp.tile([P, K], f32, tag="segsel")
                nc.vector.reduce_sum(
                    out=segsel[:, :], in_=cmp3, axis=mybir.AxisListType.X
                )

                # global candidate column = segsel*L + off, then * attention_mask
                colf = mp.tile([P, K], f32, tag="colf")
                nc.vector.tensor_scalar_mul(colf[:, :], segsel[:, :], float(L))
                nc.vector.tensor_tensor(
                    out=colf[:, :], in0=colf[:, :], in1=offf[:, :], op=Alu.add
                )
                am_t = mp.tile([P, 1], f32, tag="am")
                nc.sync.dma_start(out=am_t[:, :], in_=amf[rows, :])
                nc.vector.tensor_scalar_mul(colf[:, :], colf[:, :], am_t[:, :1])
                coli = mp.tile([P, K], i32, tag="coli")
                nc.vector.tensor_copy(out=coli[:, :], in_=colf[:, :])

                # ---- Phase F: gather embedding rows (+their sq norms) ----
                cemb = gp.tile([P, K * DP], f32, tag="cemb")
                for k in range(K):
                    nc.gpsimd.indirect_dma_start(
                        out=cemb[:, k * DP:(k + 1) * DP],
                        out_offset=None,
                        in_=embp[:, :],
                        in_offset=IndirectOffsetOnAxis(
                            ap=coli[:, k:k + 1], axis=0
                        ),
                    )

                # ---- Phase G: dot products (f32 exact via TTR) ----
                dg_t = gp.tile([P, D], f32, tag="dg")
                se_t = gp.tile([P, D], f32, tag="se")
                nc.sync.dma_start(out=dg_t[:, :], in_=dg[rows, :])
                nc.sync.dma_start(out=se_t[:, :], in_=se[rows, :])

                # dot products: per-candidate tensor_tensor mult on DVE +
                # accumulation on the (otherwise idle) ScalarE via
                # activation(Copy, accum_out) — split per k so each slot's
                # multiply starts as soon as its gather lands.
                # (InstTensorTensorReduce faults at runtime on this build.)
                newdot = mp.tile([P, K], f32, tag="newdot")
                embdot = mp.tile([P, K], f32, tag="embdot")
                prevdot = mp.tile([P, 1], f32, tag="prevdot")
                srcsq = mp.tile([P, 1], f32, tag="srcsq")
                for k in range(K):
                    erow = cemb[:, k * DP:k * DP + D]
                    for src_t, dot_out in ((dg_t, newdot), (se_t, embdot)):
                        prod = gp.tile([P, D], f32, tag="prod", bufs=6)
                        nc.vector.tensor_tensor(
                            out=prod[:, :], in0=src_t[:, :], in1=erow,
                            op=Alu.mult
                        )
                        nc.scalar.activation(
                            out=prod[:, :], in_=prod[:, :],
                            func=mybir.ActivationFunctionType.Copy,
                            accum_out=dot_out[:, k:k + 1],
                        )
                scr = gp.tile([P, D], f32, tag="scr")
                nc.vector.tensor_tensor(
                    out=scr[:, :], in0=dg_t[:, :], in1=se_t[:, :], op=Alu.mult
                )
                nc.scalar.activation(
                    out=scr[:, :], in_=scr[:, :],
                    func=mybir.ActivationFunctionType.Copy,
                    accum_out=prevdot[:, :1],
                )
                scr2 = gp.tile([P, D], f32, tag="scr2")
                nc.vector.tensor_tensor(
                    out=scr2[:, :], in0=se_t[:, :], in1=se_t[:, :], op=Alu.mult
                )
                nc.scalar.activation(
                    out=scr2[:, :], in_=scr2[:, :],
                    func=mybir.ActivationFunctionType.Copy,
                    accum_out=srcsq[:, :1],
                )
                embsq = mp.tile([P, K], f32, tag="embsq")
                nc.vector.tensor_copy(
                    out=embsq[:, :].rearrange("p (k o) -> p k o", o=1),
                    in_=cemb[:, :].rearrange("p (k d) -> p k d", d=DP)[:, :, D:D + 1],
                )

                # ---- Phase H: dir values, validity, final select ----
                d2 = mp.tile([P, K], f32, tag="d2")
                nc.vector.tensor_scalar_mul(d2[:, :], embdot[:, :], -2.0)
                nc.vector.tensor_tensor(
                    out=d2[:, :], in0=d2[:, :], in1=embsq[:, :], op=Alu.add
                )
                nc.vector.tensor_scalar_add(d2[:, :], d2[:, :], srcsq[:, :1])
                nc.vector.tensor_scalar(
                    d2[:, :], d2[:, :], 0.0, 1e-20, op0=Alu.max, op1=Alu.add
                )
                dn = mp.tile([P, K], f32, tag="dn")
                nc.scalar.sqrt(out=dn[:, :], in_=d2[:, :])
                rec = mp.tile([P, K], f32, tag="rec")
                nc.vector.reciprocal(out=rec[:, :], in_=dn[:, :])
                diff = mp.tile([P, K], f32, tag="diff")
                nc.vector.tensor_scalar(
                    diff[:, :], newdot[:, :], prevdot[:, :1], None, op0=Alu.subtract
                )
                dirv = mp.tile([P, K], f32, tag="dirv")
                nc.vector.tensor_tensor(
                    out=dirv[:, :], in0=diff[:, :], in1=rec[:, :], op=Alu.mult
                )

                tok_t = mp.tile([P, 1], f32, tag="tok")
                nc.sync.dma_start(out=tok_t[:, :], in_=tokf[rows, :])
                vge = mp.tile([P, K], f32, tag="vge")
                nc.vector.tensor_scalar(
                    vge[:, :], colf[:, :], float(NUM_SPECIAL), None, op0=Alu.is_ge
                )
                vne = mp.tile([P, K], f32, tag="vne")
                nc.vector.tensor_scalar(
                    vne[:, :], colf[:, :], tok_t[:, :1], None, op0=Alu.not_equal
                )
                validi = mp.tile([P, K], i32, tag="validi")
                nc.vector.tensor_tensor(
                    out=validi[:, :], in0=vge[:, :], in1=vne[:, :], op=Alu.mult
                )

                negk = mp.tile([P, K], f32, tag="negk")
                nc.vector.memset(negk[:, :], NEG)
                score = mp.tile([P, K], f32, tag="score")
                nc.vector.select(
                    out=score[:, :], mask=validi[:, :],
                    on_true=dirv[:, :], on_false=negk[:, :],
                )
                st8 = mp.tile([P, K], f32, tag="st8")
                nc.vector.max(out=st8[:, :], in_=score[:, :])
                match = mp.tile([P, K], f32, tag="match")
                nc.vector.tensor_scalar(
                    match[:, :], score[:, :], st8[:, :1], None, op0=Alu.is_equal
                )
                flipf = mp.tile([P, 1], f32, tag="flipf")
                mscr = mp.tile([P, K], f32, tag="mscr")
                nc.vector.tensor_tensor(
                    out=mscr[:, :], in0=match[:, :], in1=colf[:, :], op=Alu.mult
                )
                nc.vector.reduce_sum(
                    out=flipf[:, :1], in_=mscr[:, :], axis=mybir.AxisListType.X
                )
                # all-invalid rows -> flip 0
                inv = mp.tile([P, 1], f32, tag="inv")
                nc.vector.tensor_scalar(
                    inv[:, :], st8[:, :1], NEG, None, op0=Alu.not_equal
                )
                nc.vector.tensor_tensor(
                    out=flipf[:, :], in0=flipf[:, :], in1=inv[:, :], op=Alu.mult
                )

                # swap mask & final tokens
                ru_t = mp.tile([P, 1], f32, tag="ru")
                nc.sync.dma_start(out=ru_t[:, :], in_=ruf[rows, :])
                nos = mp.tile([P, 1], f32, tag="nos")
                nc.vector.tensor_scalar(
                    nos[:, :], tok_t[:, :], float(NUM_SPECIAL), None, op0=Alu.is_ge
                )
                sw = mp.tile([P, 1], f32, tag="sw")
                nc.vector.tensor_scalar(
                    sw[:, :], ru_t[:, :], SWAP_THRESH, None, op0=Alu.is_gt
                )
                mii = mp.tile([P, 1], i32, tag="mii")
                nc.vector.tensor_tensor(
                    out=mii[:, :], in0=nos[:, :], in1=sw[:, :], op=Alu.mult
                )
                advf = mp.tile([P, 1], f32, tag="advf")
                nc.vector.select(
                    out=advf[:, :], mask=mii[:, :],
                    on_true=flipf[:, :], on_false=tok_t[:, :],
                )
                nc.sync.dma_start(out=adv[rows, :], in_=advf[:, :])
    nc.compile()
    return nc


_NC_CACHE = {}


def _get_nc():
    if "nc" not in _NC_CACHE:
        _NC_CACHE["nc"] = build_nc()
    return _NC_CACHE["nc"]


def make_in_maps(delta_grad, src_embeds, embedding_matrix, src_tokens,
                 pred_lm, attention_mask, rand_u):
    pred = np.ascontiguousarray(
        np.asarray(pred_lm, dtype=np.float32).reshape(ROWS, V)
    )
    pred_pad = np.full((ROWS, VPAD), np.float32(NEG), dtype=np.float32)
    pred_pad[:, :V] = pred
    dg = np.asarray(delta_grad, dtype=np.float32).reshape(ROWS, D)
    se = np.asarray(src_embeds, dtype=np.float32).reshape(ROWS, D)
    emb = np.asarray(embedding_matrix, dtype=np.float32)
    embsq = np.einsum("vd,vd->v", emb, emb, dtype=np.float32).astype(np.float32)
    embp = np.ascontiguousarray(
        np.concatenate([emb, embsq[:, None]], axis=1).astype(np.float32)
    )
    tokf = np.asarray(src_tokens).reshape(ROWS, 1).astype(np.float32)
    amf = np.asarray(attention_mask).reshape(ROWS, 1).astype(np.float32)
    ruf = np.asarray(rand_u, dtype=np.float32).reshape(ROWS, 1)
    rbf = (np.arange(R, dtype=np.float32) * G).reshape(R, 1)

    in_maps = []
    for c in range(N_CORES):
        sl = slice(c * R, (c + 1) * R)
        in_maps.append({
            "pred": np.ascontiguousarray(pred_pad[sl]),
            "dg": np.ascontiguousarray(dg[sl]),
            "se": np.ascontiguousarray(se[sl]),
            "embp": embp,
            "tokf": np.ascontiguousarray(tokf[sl]),
            "ruf": np.ascontiguousarray(ruf[sl]),
            "amf": np.ascontiguousarray(amf[sl]),
            "rbf": rbf,
        })
    return in_maps


def run_cores(in_maps, trace=False):
    from concourse.bass_utils import run_bass_kernel_spmd
    nc = _get_nc()
    return run_bass_kernel_spmd(
        nc, in_maps, core_ids=list(range(N_CORES)), trace=trace
    )


def kernel(delta_grad, src_embeds, embedding_matrix, src_tokens, pred_lm,
           attention_mask, rand_u):
    in_maps = make_in_maps(delta_grad, src_embeds, embedding_matrix,
                           src_tokens, pred_lm, attention_mask, rand_u)
    res = run_cores(in_maps, trace=False)
    advs = [res.results[c]["adv"].reshape(R) for c in range(N_CORES)]
    out = np.concatenate(advs).reshape(B, S)
    return out.astype(np.asarray(src_tokens).dtype)

